# revision 1
# baseline (speedup 1.0000x reference)
"""Bass/Trainium2 kernel for nn_Attn_70076686401576 (block-causal-biased MHA).

Math (per reference):
  qkv = x @ Wqkv + bqkv  -> split into q,k,v heads (H=16, hd=64)
  q,k RMS-normalized over head dim (QKNorm, eps=1e-6, scales gq/gk)
  scores = q k^T / sqrt(hd) + M, where M[i,j] = 1.0 for future-frame keys
  attn = softmax(scores); o = attn @ v; out = o @ Wout + bout

Sharding: 16 heads / 8 cores = 2 heads per core (head-parallel).  Each core
computes its 2 heads' q/k/v from the full x (Wqkv column-sharded), runs full
attention for those heads, and produces a partial output via the row-sharded
Wout.  Host sums the 8 partials (+ bout).

Layout/precision tricks:
  - projection computed in transposed layout qkv^T = W^T x^T (f32r) so the
    head dim lands on SBUF partitions; x^T tiles made with PE transposes
  - RMS sum-of-squares over the partition dim via a block-diag ones matmul,
    which also broadcasts the per-token sums to all 64 partitions of a head
  - the projection/QKNorm chain runs in f32r (FP22); the normalized q,k are
    emitted in bf16 by the final RMS multiply, so the two big attention
    matmuls (scores, attn@V) stream bf16 at full PE rate
  - the "+1.0 future-frame" mask folded into the attn@v matmul by keeping two
    copies of V (V and e*V): exp(S+1) V = exp(S) (eV)
  - softmax denominator via a ones-column appended to V
  - output projection in f32r
"""

import math
import numpy as np

N_TOK_FULL = 4096
D_MODEL = 1024
HD = 64
TPF = 256
EPS = 1e-6
N_CORES = 8


def build_program(n_tok=N_TOK_FULL, debug=False):
    import concourse.bass as bass
    import concourse.tile as tile
    from concourse import bacc, mybir
    from concourse.masks import make_identity
    from contextlib import ExitStack

    f32 = mybir.dt.float32
    f32r = mybir.dt.float32r
    bf16 = mybir.dt.bfloat16
    AF = mybir.ActivationFunctionType
    E_CONST = float(np.exp(1.0))

    D = D_MODEL
    n_ranges = n_tok // 512       # 512-token ranges (projection / q stripes)
    n_ktiles = n_tok // 128
    n_stripes = n_tok // 512
    n_frames = n_tok // TPF       # frames == key groups of 2 ktiles

    nc = bacc.Bacc("TRN2", target_bir_lowering=False, debug=False,
                   num_devices=N_CORES)
    x_d = nc.dram_tensor("x", [n_tok, D], f32, kind="ExternalInput").ap()
    wqkv_d = nc.dram_tensor("wqkv", [D, 384], f32, kind="ExternalInput").ap()
    bqkv_d = nc.dram_tensor("bqkv", [384], f32, kind="ExternalInput").ap()
    gv_d = nc.dram_tensor("gv", [128, 2], f32, kind="ExternalInput").ap()
    wout_d = nc.dram_tensor("wout", [128, D], f32, kind="ExternalInput").ap()
    out_d = nc.dram_tensor("out", [n_tok, D], f32, kind="ExternalOutput").ap()
    dbg = {}
    if debug:
        for nm, shp in (("dbg_qT", [128, n_tok]), ("dbg_kT", [128, n_tok]),
                        ("dbg_vT", [128, n_tok]),
                        ("dbg_oTn0", [64, n_tok]), ("dbg_oTn1", [64, n_tok]),
                        ("dbg_po", [65, 512])):
            dbg[nm] = nc.dram_tensor(nm, shp, f32, kind="ExternalOutput").ap()

    x_t = x_d.rearrange("(t p) d -> t p d", p=128)
    out_t = out_d.rearrange("(t p) d -> t p d", p=128)

    with tile.TileContext(nc) as tc:
        ctx = ExitStack()
        sb = ctx.enter_context(tc.tile_pool(name="sb", bufs=1))
        ps1_ctx = ExitStack()
        ps1 = ps1_ctx.enter_context(
            tc.tile_pool(name="ps1", bufs=1, space="PSUM"))
        sbp_ctx = ExitStack()
        sbp = sbp_ctx.enter_context(tc.tile_pool(name="sbp", bufs=1))
        if True:
            # ---- constants ----
            identf = sb.tile([128, 128], f32, tag="identf")
            make_identity(nc, identf)
            ident = sb.tile([128, 128], f32r, tag="ident")
            nc.vector.tensor_copy(ident, identf)
            identb = sb.tile([128, 128], bf16, tag="identb")
            nc.vector.tensor_copy(identb, identf)
            # block-diag ones: out = blkdiag.T @ sq gives per-head column sums
            # broadcast to that head's 64 partitions
            blkdf = sb.tile([128, 128], f32, tag="blkdf")
            nc.gpsimd.memset(blkdf, 0.0)
            nc.gpsimd.memset(blkdf[0:64, 0:64], 1.0)
            nc.gpsimd.memset(blkdf[64:128, 64:128], 1.0)
            blkdiag = sb.tile([128, 128], f32r, tag="blkdiag")
            nc.vector.tensor_copy(blkdiag, blkdf)
            ones64 = sb.tile([128, 64], f32, tag="ones64")
            nc.gpsimd.memset(ones64, 1.0)
            cb_q = sb.tile([128, 1], f32, tag="cb_q")
            nc.gpsimd.memset(cb_q, 64.0 * EPS)
            cb_k = sb.tile([128, 1], f32, tag="cb_k")
            nc.gpsimd.memset(cb_k, EPS)
            cs_k = sb.tile([128, 1], f32, tag="cs_k")
            nc.gpsimd.memset(cs_k, 1.0 / 64.0)

            wqkvf = sb.tile([128, 8, 384], f32, tag="wqkvf")
            nc.sync.dma_start(wqkvf,
                              wqkv_d.rearrange("(c p) n -> p c n", p=128))
            wqkv_sb = sb.tile([128, 8, 384], bf16, tag="wqkv")
            nc.vector.tensor_copy(wqkv_sb, wqkvf)
            bq_sb = sb.tile([128, 3], f32, tag="bq")
            nc.sync.dma_start(bq_sb, bqkv_d.rearrange("(c p) -> p c", p=128))
            gv_sb = sb.tile([128, 2], f32, tag="gv")
            nc.sync.dma_start(gv_sb, gv_d)
            wof = sb.tile([128, D], f32, tag="wof")
            nc.sync.dma_start(wof, wout_d)
            wo0 = sb.tile([64, D], bf16, tag="wo0")
            nc.vector.tensor_copy(wo0, wof[0:64, :])
            wo1 = sb.tile([64, D], bf16, tag="wo1")
            nc.vector.tensor_copy(wo1, wof[64:128, :])

            # ---- persistent blocks ----
            # qkv^T layout: partition = 2*64 head dims (h0 rows 0:64, h1 64:128)
            qT = sb.tile([128, n_tok], f32r, tag="qT")
            kT = sb.tile([128, n_tok], f32r, tag="kT")
            vT = sb.tile([128, n_tok], f32r, tag="vT")
            qTb = sb.tile([128, n_tok], bf16, tag="qTb")   # normalized, bf16
            kTb = sb.tile([128, n_tok], bf16, tag="kTb")
            oTn0 = sb.tile([64, n_tok], bf16, tag="oTn0")
            oTn1 = sb.tile([64, n_tok], bf16, tag="oTn1")
            # V in natural layout + ones column (denominator), per head,
            # plus the e-scaled copies for the future-frame mask (bf16)
            va0 = sb.tile([128, n_ktiles, 65], bf16, tag="va0")
            va1 = sb.tile([128, n_ktiles, 65], bf16, tag="va1")
            eva0 = sb.tile([128, n_ktiles, 65], bf16, tag="eva0")
            eva1 = sb.tile([128, n_ktiles, 65], bf16, tag="eva1")

            # ================= phase 1: projection + QKNorm =================
            for r in range(n_ranges):
                xTr = sbp.tile([128, 8, 512], bf16, tag="xT", bufs=3)
                for tt in range(4):
                    gt = r * 4 + tt
                    xinf = sbp.tile([128, D], f32, tag="xinf", bufs=4)
                    # two dma_starts -> two DMA queues, 2x stream bandwidth
                    nc.sync.dma_start(xinf[0:64, :], x_t[gt][0:64, :])
                    nc.sync.dma_start(xinf[64:128, :], x_t[gt][64:128, :])
                    xin = sbp.tile([128, D], bf16, tag="xin", bufs=4)
                    # cast on the (otherwise idle) scalar engine
                    nc.scalar.copy(xin, xinf)
                    for dc in range(8):
                        ps_xp = ps1.tile([128, 128], bf16, tag="xp", bufs=2)
                        nc.tensor.transpose(ps_xp,
                                            xin[:, dc * 128:(dc + 1) * 128],
                                            identb)
                        nc.vector.tensor_copy(
                            xTr[:, dc, tt * 128:(tt + 1) * 128],
                            ps_xp)

                pj = []
                for oc in range(3):
                    pj_oc = ps1.tile([128, 512], f32, tag=f"pj{oc}", bufs=2)
                    pj.append(pj_oc)
                for dc in range(8):
                    for oc in range(3):
                        nc.tensor.matmul(
                            pj[oc],
                            wqkv_sb[:, dc, oc * 128:(oc + 1) * 128],
                            xTr[:, dc, :],
                            start=(dc == 0), stop=(dc == 7))
                sl = slice(r * 512, (r + 1) * 512)
                for oc, blk in ((0, qT), (1, kT), (2, vT)):
                    nc.vector.tensor_scalar_add(blk[:, sl], pj[oc],
                                                bq_sb[:, oc:oc + 1])

                # QKNorm: rsqrt(mean(q^2) + eps); the 1/sqrt(hd)=0.125 score
                # scale is folded into the q branch via sqrt(sumsq + 64*eps).
                # The final multiply emits the bf16 copies used by attention.
                for which, blk, blkb in (("q", qT, qTb), ("k", kT, kTb)):
                    sq = sbp.tile([128, 512], f32r, tag="sq", bufs=2)
                    nc.scalar.activation(sq, blk[:, sl], AF.Square)
                    ps_r = ps1.tile([128, 512], f32, tag="xp", bufs=2, name=f"psr_{r}_{which}")
                    nc.tensor.matmul(ps_r, blkdiag, sq,
                                     start=True, stop=True)
                    sqs = sbp.tile([128, 512], f32, tag="sqs", bufs=2)
                    if which == "q":
                        nc.scalar.activation(sqs, ps_r, AF.Sqrt,
                                             bias=cb_q, scale=1.0)
                    else:
                        nc.scalar.activation(sqs, ps_r, AF.Sqrt,
                                             bias=cb_k, scale=cs_k)
                    rs = sbp.tile([128, 512], f32, tag="rs", bufs=2)
                    nc.vector.reciprocal_approx_fast(rs, sqs)
                    gcol = 0 if which == "q" else 1
                    nc.vector.tensor_scalar_mul(rs, rs,
                                                gv_sb[:, gcol:gcol + 1])
                    nc.vector.tensor_mul(blkb[:, sl], blk[:, sl], rs)

                # V -> Va/eVa for this range's 4 ktiles (overlaps the next
                # range's projection instead of a serial phase at the end)
                for kt in range(4 * r, 4 * r + 4):
                    ps_vt = ps1.tile([128, 128], f32r, tag="xp", bufs=2,
                                     name=f"psvt_{kt}")
                    nc.tensor.transpose(ps_vt,
                                        vT[:, kt * 128:(kt + 1) * 128],
                                        ident)
                    nc.vector.tensor_copy(va0[:, kt, 0:64], ps_vt[:, 0:64])
                    nc.vector.tensor_copy(va1[:, kt, 0:64], ps_vt[:, 64:128])
                    nc.vector.tensor_copy(va0[:, kt, 64:65], ones64[:, 0:1])
                    nc.vector.tensor_copy(va1[:, kt, 64:65], ones64[:, 0:1])
                    nc.vector.tensor_scalar_mul(eva0[:, kt, :],
                                                va0[:, kt, :], E_CONST)
                    nc.vector.tensor_scalar_mul(eva1[:, kt, :],
                                                va1[:, kt, :], E_CONST)

            if debug:
                nc.sync.dma_start(dbg["dbg_qT"], qT.bitcast(f32))
                nc.sync.dma_start(dbg["dbg_kT"], kT.bitcast(f32))
                nc.sync.dma_start(dbg["dbg_vT"], vT.bitcast(f32))

            # ================= phase 3: attention =================
            sbp_ctx.close()
            ps1_ctx.close()
            ps2_ctx = ExitStack()
            ps2 = ps2_ctx.enter_context(
                tc.tile_pool(name="ps2", bufs=1, space="PSUM"))
            sba_ctx = ExitStack()
            sba = sba_ctx.enter_context(tc.tile_pool(name="sba", bufs=1))

            vab = (va0, va1)
            evab = (eva0, eva1)

            def emit_norm_outproj(s, po_s):
                """Normalize stripe s (divide by denom row) + output proj."""
                qsl = slice(s * 512, (s + 1) * 512)
                for h in range(2):
                    rd = sba.tile([65, 512], f32, tag="rd", bufs=3,
                                  name=f"rd_{s}_{h}")
                    # custom-DVE ops misbehave at base_partition != 0: compute
                    # recip over all 65 rows from base 0; only row 64 (the
                    # denominator row) is consumed by the broadcast matmul
                    nc.vector.reciprocal_approx_fast(rd, po_s[h])
                    ps_b = ps2.tile([64, 512], f32, tag="sg", bufs=3,
                                    name=f"psb_{s}_{h}")
                    nc.tensor.matmul(ps_b,
                                     ones64[64:65, :],
                                     rd[64:65, :],
                                     start=True, stop=True,
                                     tile_position=(64, 0))
                    rb = sba.tile([64, 512], f32, tag="rb", bufs=3,
                                  name=f"rb_{s}_{h}")
                    nc.vector.tensor_copy(rb, ps_b)
                    if debug and s == 0 and h == 0:
                        po_cp = sba.tile([65, 512], f32, tag="po_cp")
                        nc.vector.tensor_copy(po_cp, po_s[0])
                        nc.sync.dma_start(dbg["dbg_po"], po_cp)
                    oTn = (oTn0, oTn1)[h]
                    nc.vector.tensor_mul(oTn[:, qsl], po_s[h][0:64, :], rb)
                for tt in range(4):
                    t0 = s * 512 + tt * 128
                    ps_o = ps2.tile([128, 1024], f32, tag="sg", bufs=3,
                                    name=f"pso_{s}_{tt}")
                    for half in range(2):
                        nsl = slice(half * 512, (half + 1) * 512)
                        nc.tensor.matmul(ps_o[:, nsl],
                                         oTn0[:, t0:t0 + 128],
                                         wo0[:, nsl],
                                         start=True, stop=False)
                        nc.tensor.matmul(ps_o[:, nsl],
                                         oTn1[:, t0:t0 + 128],
                                         wo1[:, nsl],
                                         start=False, stop=True)
                    ob = sba.tile([128, D], f32, tag="ob", bufs=4,
                                  name=f"ob_{s}_{tt}")
                    nc.vector.tensor_copy(ob, ps_o)
                    nc.sync.dma_start(out_t[t0 // 128], ob)

            KG = 2  # ktiles per exp batch
            kgroups = [list(range(a, min(a + KG, n_ktiles)))
                       for a in range(0, n_ktiles, KG)]
            pending = None
            for s in range(n_stripes):
                qsl = slice(s * 512, (s + 1) * 512)
                po = [ps2.tile([65, 512], f32, tag=f"op{h}", bufs=1,
                               name=f"po{h}_{s}")
                      for h in range(2)]
                for gi, kts in enumerate(kgroups):
                    ng = len(kts)
                    sg = []
                    et = []
                    for h in range(2):
                        sg_h = ps2.tile([128, ng * 512], f32, tag="sg",
                                        bufs=3, name=f"sg{h}_{s}_{gi}")
                        sg.append(sg_h)
                    # adjacent heads -> disjoint PE row groups -> concurrent
                    for i, kt in enumerate(kts):
                        for h in range(2):
                            hp = slice(h * 64, (h + 1) * 64)
                            nc.tensor.matmul(
                                sg[h][:, i * 512:(i + 1) * 512],
                                kTb[hp, kt * 128:(kt + 1) * 128],
                                qTb[hp, qsl],
                                start=True, stop=True,
                                tile_position=(h * 64, 0))
                    for h in range(2):
                        et_h = sba.tile([128, KG * 512], bf16, tag="et",
                                        bufs=6, name=f"et{h}_{s}_{gi}")
                        et.append(et_h)
                        nc.scalar.activation(et_h[:, 0:ng * 512], sg[h],
                                             AF.Exp)
                    for h in range(2):
                        for i, kt in enumerate(kts):
                            fk = kt // 2
                            rhs = et[h]
                            first = (kt == 0)
                            last = (kt == n_ktiles - 1)
                            if fk == 2 * s + 1:
                                # key frame == 2nd query frame of the stripe:
                                # first 256 queries see it as future (e*V)
                                nc.tensor.matmul(
                                    po[h][:, 0:256],
                                    evab[h][:, kt, :],
                                    rhs[:, i * 512:i * 512 + 256],
                                    start=False, stop=False)
                                # stop only on the final matmul (the whole
                                # [65,512] tile is one 2KB psum zero region)
                                nc.tensor.matmul(
                                    po[h][:, 256:512],
                                    vab[h][:, kt, :],
                                    rhs[:, i * 512 + 256:(i + 1) * 512],
                                    start=False, stop=last)
                            else:
                                vv = evab[h] if fk > 2 * s + 1 else vab[h]
                                nc.tensor.matmul(
                                    po[h][:, :],
                                    vv[:, kt, :],
                                    rhs[:, i * 512:(i + 1) * 512],
                                    start=first, stop=last)
                    if pending is not None and gi == 2:
                        emit_norm_outproj(*pending)
                        pending = None
                pending = (s, po)
            emit_norm_outproj(*pending)
            if debug:
                nc.sync.dma_start(dbg["dbg_oTn0"], oTn0.bitcast(f32))
                nc.sync.dma_start(dbg["dbg_oTn1"], oTn1.bitcast(f32))

            sba_ctx.close()
            ps2_ctx.close()
            ctx.close()

    nc.compile()
    return nc


def shard_inputs(x, Wqkv, bqkv, gq, gk, Wout, n_tok):
    """Build the 8 per-core input maps (head-parallel sharding)."""
    D = D_MODEL
    in_maps = []
    for c in range(N_CORES):
        cs = slice(128 * c, 128 * (c + 1))
        wq = Wqkv[:, cs]
        wk = Wqkv[:, D + 128 * c:D + 128 * (c + 1)]
        wv = Wqkv[:, 2 * D + 128 * c:2 * D + 128 * (c + 1)]
        wqkv_s = np.ascontiguousarray(np.concatenate([wq, wk, wv], axis=1),
                                      dtype=np.float32)
        bq = bqkv[cs]
        bk = bqkv[D + 128 * c:D + 128 * (c + 1)]
        bv = bqkv[2 * D + 128 * c:2 * D + 128 * (c + 1)]
        bqkv_s = np.ascontiguousarray(np.concatenate([bq, bk, bv]),
                                      dtype=np.float32)
        gv = np.stack([np.concatenate([gq, gq]),
                       np.concatenate([gk, gk])], axis=1).astype(np.float32)
        wout_s = np.ascontiguousarray(Wout[cs, :], dtype=np.float32)
        in_maps.append({
            "x": np.ascontiguousarray(x[:n_tok], dtype=np.float32),
            "wqkv": wqkv_s,
            "bqkv": bqkv_s,
            "gv": np.ascontiguousarray(gv),
            "wout": wout_s,
        })
    return in_maps


_PROGRAM_CACHE = {}


def _get_program(n_tok):
    if n_tok not in _PROGRAM_CACHE:
        _PROGRAM_CACHE[n_tok] = build_program(n_tok)
    return _PROGRAM_CACHE[n_tok]


def run_sharded(inputs, trace=False, tmpdir=None):
    """Run the SPMD kernel; returns (full_output [1,N,D], BassKernelResults)."""
    from concourse.bass_utils import run_bass_kernel_spmd

    x = np.asarray(inputs["x"], dtype=np.float32)
    Wqkv = np.asarray(inputs["Wqkv"], dtype=np.float32)
    bqkv = np.asarray(inputs["bqkv"], dtype=np.float32)
    Wout = np.asarray(inputs["Wout"], dtype=np.float32)
    bout = np.asarray(inputs["bout"], dtype=np.float32)
    gq = np.asarray(inputs["gq"], dtype=np.float32)
    gk = np.asarray(inputs["gk"], dtype=np.float32)
    tpf = int(np.asarray(inputs["tokens_per_frame"]))
    assert tpf == TPF, f"kernel hardcodes tokens_per_frame={TPF}, got {tpf}"

    B, N, D = x.shape
    assert B == 1 and D == D_MODEL
    x2 = x[0]

    nc = _get_program(N)
    in_maps = shard_inputs(x2, Wqkv, bqkv, gq, gk, Wout, N)
    res = run_bass_kernel_spmd(nc, in_maps, list(range(N_CORES)),
                               trace=trace, tmpdir=tmpdir)
    acc = res.results[0]["out"].astype(np.float32)
    for c in range(1, N_CORES):
        acc = acc + res.results[c]["out"]
    if np.any(bout):
        acc = acc + bout[None, :]
    return acc[None], res


def kernel(**inputs):
    out, _ = run_sharded(inputs)
    return out



# revision 5
# speedup vs baseline: 1.2447x; 1.2447x over previous
"""Bass/Trainium2 kernel for nn_Attn_70076686401576 (block-causal-biased MHA).

Math (per reference):
  qkv = x @ Wqkv + bqkv  -> split into q,k,v heads (H=16, hd=64)
  q,k RMS-normalized over head dim (QKNorm, eps=1e-6, scales gq/gk)
  scores = q k^T / sqrt(hd) + M, where M[i,j] = 1.0 for future-frame keys
  attn = softmax(scores); o = attn @ v; out = o @ Wout + bout
Sharding: 16 heads / 8 cores = 2 heads per core (head-parallel).  Each core
computes its 2 heads' q/k/v from the full x (Wqkv column-sharded), runs full
attention for those heads, and produces a partial output via the row-sharded
Wout.  Host sums the 8 partials (+ bout).

Key structure (v2):
  - phase 1: x tiles DMA'd f32, cast to bf16 on GpSimd, transposed on PE
    (grouped 4-at-a-time into PSUM, copied out alternately by Scalar/Vector),
    projected with bf16 weights; QKNorm folds the 1/sqrt(hd) score scale and
    the gq/gk scales into the Sqrt activation's per-partition scale/bias.
  - phase 2 is a per-ktile software pipeline: the two heads' score matmuls
    run concurrently on PE row-halves into adjacent PSUM banks; the combined
    [128,2,512] tile is exponentiated by ONE engine op, alternating between
    ScalarE (exact ACT Exp) and DVE (Schraudolph bit-trick exp: one affine
    tensor_scalar emitting int16 bf16-bit-patterns) so the two engines share
    the exp load; attn@V accumulates into per-stripe PSUM; the "+1.0
    future-frame" mask is folded into the exp (ACT bias=+1 / Schraudolph
    +A_SCH) so no e-scaled V copy is needed.
  - softmax denominator via a ones-column appended to V; normalization via
    reciprocal + PE row-broadcast; output projection from bf16 oTn.
"""

import math
import numpy as np

N_TOK_FULL = 4096
D_MODEL = 1024
HD = 64
TPF = 256
EPS = 1e-6
N_CORES = 8

LN2 = math.log(2.0)
SCH_A = 128.0 / LN2            # bf16 Schraudolph multiplier
SCH_C = -6.0                   # balance constant (minimizes max rel err)
SCH_B = 127.0 * 128.0 + SCH_C


def build_program(n_tok=N_TOK_FULL, debug=False):
    import concourse.bass as bass
    import concourse.tile as tile
    from concourse import bacc, mybir
    from concourse.masks import make_identity
    from contextlib import ExitStack

    f32 = mybir.dt.float32
    f32r = mybir.dt.float32r
    bf16 = mybir.dt.bfloat16
    i16 = mybir.dt.int16
    AF = mybir.ActivationFunctionType
    MUL = mybir.AluOpType.mult
    ADD = mybir.AluOpType.add

    D = D_MODEL
    n_ranges = n_tok // 512
    n_ktiles = n_tok // 128
    n_stripes = n_tok // 512

    nc = bacc.Bacc("TRN2", target_bir_lowering=False, debug=False,
                   num_devices=N_CORES)
    x_d = nc.dram_tensor("x", [n_tok, D], f32, kind="ExternalInput").ap()
    wqkv_d = nc.dram_tensor("wqkv", [D, 384], f32, kind="ExternalInput").ap()
    bqkv_d = nc.dram_tensor("bqkv", [384], f32, kind="ExternalInput").ap()
    # gv: per-partition [scale_q, bias_q, scale_k, bias_k] for the RMS sqrt
    gv_d = nc.dram_tensor("gv", [128, 4], f32, kind="ExternalInput").ap()
    wout_d = nc.dram_tensor("wout", [128, D], f32, kind="ExternalInput").ap()
    out_d = nc.dram_tensor("out", [n_tok, D], f32, kind="ExternalOutput").ap()

    x_t = x_d.rearrange("(t p) d -> t p d", p=128)
    out_t = out_d.rearrange("(t p) d -> t p d", p=128)

    with tile.TileContext(nc) as tc:
        ctx = ExitStack()
        sb = ctx.enter_context(tc.tile_pool(name="sb", bufs=1))
        ps1_ctx = ExitStack()
        ps1 = ps1_ctx.enter_context(
            tc.tile_pool(name="ps1", bufs=1, space="PSUM"))
        sbp_ctx = ExitStack()
        sbp = sbp_ctx.enter_context(tc.tile_pool(name="sbp", bufs=1))

        # ---- constants ----
        identf = sb.tile([128, 128], f32, tag="identf")
        make_identity(nc, identf)
        identb = sb.tile([128, 128], bf16, tag="identb")
        nc.vector.tensor_copy(identb, identf)
        # block-diag ones: blkdiag.T @ sq -> per-head column sums broadcast
        # to that head's 64 partitions
        blkdf = sb.tile([128, 128], f32, tag="blkdf")
        nc.gpsimd.memset(blkdf, 0.0)
        nc.gpsimd.memset(blkdf[0:64, 0:64], 1.0)
        nc.gpsimd.memset(blkdf[64:128, 64:128], 1.0)
        blkdiag = sb.tile([128, 128], f32r, tag="blkdiag")
        nc.vector.tensor_copy(blkdiag, blkdf)
        ones64 = sb.tile([128, 64], f32, tag="ones64")
        nc.gpsimd.memset(ones64, 1.0)

        wqkvf = sb.tile([128, 8, 384], f32, tag="wqkvf")
        nc.sync.dma_start(wqkvf, wqkv_d.rearrange("(c p) n -> p c n", p=128))
        wqkv_sb = sb.tile([128, 8, 384], bf16, tag="wqkv")
        nc.vector.tensor_copy(wqkv_sb, wqkvf)
        bq_sb = sb.tile([128, 3], f32, tag="bq")
        nc.sync.dma_start(bq_sb, bqkv_d.rearrange("(c p) -> p c", p=128))
        gv_sb = sb.tile([128, 4], f32, tag="gv")
        nc.sync.dma_start(gv_sb, gv_d)
        wof = sb.tile([128, D], f32, tag="wof")
        nc.sync.dma_start(wof, wout_d)
        wo0 = sb.tile([64, D], bf16, tag="wo0")
        nc.vector.tensor_copy(wo0, wof[0:64, :])
        wo1 = sb.tile([64, D], bf16, tag="wo1")
        nc.vector.tensor_copy(wo1, wof[64:128, :])

        # ---- persistent blocks ----
        qTb = sb.tile([128, n_tok], bf16, tag="qTb")   # normalized q^T
        kTb = sb.tile([128, n_tok], bf16, tag="kTb")
        oTn0 = sb.tile([64, n_tok], bf16, tag="oTn0")
        oTn1 = sb.tile([64, n_tok], bf16, tag="oTn1")
        # V natural layout per (ktile, head): [keys=128, kt, h, hd+ones]
        va = sb.tile([128, n_ktiles, 2, 65], bf16, tag="va")
        nc.gpsimd.memset(va[:, :, :, 64:65], 1.0)

        # ================= phase 1: projection + QKNorm =================
        for r in range(n_ranges):
            xTr = sbp.tile([128, 8, 512], bf16, tag="xT", bufs=2)
            for tt in range(4):
                gt = r * 4 + tt
                xinf = sbp.tile([128, D], f32, tag="xinf", bufs=3)
                nc.sync.dma_start(xinf[0:64, :], x_t[gt][0:64, :])
                nc.sync.dma_start(xinf[64:128, :], x_t[gt][64:128, :])
                xin = sbp.tile([128, D], bf16, tag="xin", bufs=3)
                nc.gpsimd.tensor_copy(xin, xinf)
                xp = ps1.tile([128, 8, 128], bf16, tag="xp", bufs=3,
                              name=f"xp_{gt}")
                for dc in range(8):
                    nc.tensor.transpose(
                        xp[:, dc, :], xin[:, dc * 128:(dc + 1) * 128],
                        identb)
                dst = xTr[:, :, tt * 128:(tt + 1) * 128]
                if tt % 2 == 0:
                    nc.scalar.copy(dst, xp)
                else:
                    nc.vector.tensor_copy(dst, xp)

            pj = ps1.tile([128, 3, 512], f32, tag="pj", bufs=1, name=f"pj_{r}")
            for dc in range(8):
                for oc in range(3):
                    nc.tensor.matmul(
                        pj[:, oc, :],
                        wqkv_sb[:, dc, oc * 128:(oc + 1) * 128],
                        xTr[:, dc, :],
                        start=(dc == 0), stop=(dc == 7))
            sl = slice(r * 512, (r + 1) * 512)
            qTr = sbp.tile([128, 512], f32r, tag="qTr", bufs=2)
            kTr = sbp.tile([128, 512], f32r, tag="kTr", bufs=2)
            vTr = sbp.tile([128, 512], bf16, tag="vTr", bufs=2)
            nc.vector.tensor_scalar_add(qTr, pj[:, 0, :], bq_sb[:, 0:1])
            nc.vector.tensor_scalar_add(kTr, pj[:, 1, :], bq_sb[:, 1:2])
            nc.vector.tensor_scalar_add(vTr, pj[:, 2, :], bq_sb[:, 2:3])

            # QKNorm: rs = g * rsqrt(mean(t^2) + eps); q additionally folds
            # the 1/sqrt(hd) score scale.  scale/bias of the Sqrt activation
            # are per-partition host-precomputed: sqs = sqrt(sumsq*s + b),
            # rs = 1/sqs.
            for blk, blkb, gcol in ((qTr, qTb, 0), (kTr, kTb, 1)):
                sq = sbp.tile([128, 512], f32r, tag="sq", bufs=2)
                nc.scalar.activation(sq, blk, AF.Square)
                ps_r = ps1.tile([128, 512], f32, tag="psr", bufs=2,
                                name=f"psr_{r}_{gcol}")
                nc.tensor.matmul(ps_r, blkdiag, sq, start=True, stop=True)
                sqs = sbp.tile([128, 512], f32, tag="sqs", bufs=2)
                nc.scalar.activation(sqs, ps_r, AF.Sqrt,
                                     bias=gv_sb[:, 2 * gcol + 1:2 * gcol + 2],
                                     scale=gv_sb[:, 2 * gcol:2 * gcol + 1])
                rs = sbp.tile([128, 512], f32, tag="rs", bufs=2)
                nc.vector.reciprocal_approx_fast(rs, sqs)
                nc.vector.tensor_mul(blkb[:, sl], blk, rs)

            # V -> va for this range's 4 ktiles
            vp = ps1.tile([128, 8, 128], bf16, tag="xp", bufs=3,
                          name=f"vp_{r}")
            for q in range(4):
                nc.tensor.transpose(
                    vp[:, q, :], vTr[:, q * 128:(q + 1) * 128], identb)
            src = vp[:, 0:4, :].rearrange("p k (h d) -> p k h d", h=2)
            if r % 2 == 0:
                nc.scalar.copy(va[:, 4 * r:4 * r + 4, :, 0:64], src)
            else:
                nc.vector.tensor_copy(va[:, 4 * r:4 * r + 4, :, 0:64], src)

        # ================= phase 2: attention =================
        sbp_ctx.close()
        ps1_ctx.close()
        ps2_ctx = ExitStack()
        ps2 = ps2_ctx.enter_context(
            tc.tile_pool(name="ps2", bufs=1, space="PSUM"))
        sba_ctx = ExitStack()
        sba = sba_ctx.enter_context(tc.tile_pool(name="sba", bufs=1))

        def emit_norm_outproj(s, po):
            """Normalize stripe s (divide by denom row) + output proj."""
            qsl = slice(s * 512, (s + 1) * 512)
            rd = sba.tile([65, 2, 512], f32, tag="rd", bufs=2,
                          name=f"rd_{s}")
            # custom-DVE ops misbehave at base_partition != 0: compute recip
            # over all 65 rows from base 0; only row 64 (the denominator row)
            # is consumed by the broadcast matmul
            nc.vector.reciprocal_approx_fast(rd, po)
            ps_b = ps2.tile([128, 2, 512], f32, tag="sg", bufs=3,
                            name=f"psb_{s}")
            for h in range(2):
                nc.tensor.matmul(ps_b[0:64, h, :],
                                 ones64[64:65, :],
                                 rd[64:65, h, :],
                                 start=True, stop=True,
                                 tile_position=(64, 0))
            rb = sba.tile([64, 2, 512], f32, tag="rb", bufs=2,
                          name=f"rb_{s}")
            nc.scalar.copy(rb, ps_b[0:64, :, :])
            for h, oTn in ((0, oTn0), (1, oTn1)):
                nc.vector.tensor_mul(oTn[:, qsl], po[0:64, h, :],
                                     rb[:, h, :])
            for tt in range(4):
                t0 = s * 512 + tt * 128
                ps_o = ps2.tile([128, 2, 512], f32, tag="sg", bufs=3,
                                name=f"pso_{s}_{tt}")
                for half in range(2):
                    nsl = slice(half * 512, (half + 1) * 512)
                    nc.tensor.matmul(ps_o[:, half, :],
                                     oTn0[:, t0:t0 + 128],
                                     wo0[:, nsl],
                                     start=True, stop=False)
                    nc.tensor.matmul(ps_o[:, half, :],
                                     oTn1[:, t0:t0 + 128],
                                     wo1[:, nsl],
                                     start=False, stop=True)
                ob = sba.tile([128, 2, 512], f32, tag="ob", bufs=3,
                              name=f"ob_{s}_{tt}")
                if tt % 2 == 0:
                    nc.scalar.copy(ob, ps_o)
                else:
                    nc.vector.tensor_copy(ob, ps_o)
                nc.sync.dma_start(out_t[t0 // 128], ob)

        pending = None
        for s in range(n_stripes):
            qsl = slice(s * 512, (s + 1) * 512)
            po = ps2.tile([65, 2, 512], f32, tag="po", bufs=1,
                          name=f"po_{s}")
            prev = None
            for kt in range(n_ktiles):
                sg = ps2.tile([128, 2, 512], f32, tag="sg", bufs=3,
                              name=f"sg_{s}_{kt}")
                for h in range(2):
                    hp = slice(h * 64, (h + 1) * 64)
                    nc.tensor.matmul(
                        sg[:, h, :],
                        kTb[hp, kt * 128:(kt + 1) * 128],
                        qTb[hp, qsl],
                        start=True, stop=True,
                        tile_position=(h * 64, 0))
                # exp (mask folded in): key frame fk vs query frames
                # (2s, 2s+1): future -> +1.0 bias
                et = sba.tile([128, 2, 512], bf16, tag="et", bufs=5,
                              name=f"et_{s}_{kt}")
                fk = kt // 2
                if kt % 2 == 0:
                    # ScalarE exact exp
                    if fk == 2 * s + 1:
                        nc.scalar.activation(et[:, :, 0:256],
                                             sg[:, :, 0:256], AF.Exp,
                                             bias=1.0)
                        nc.scalar.activation(et[:, :, 256:512],
                                             sg[:, :, 256:512], AF.Exp)
                    else:
                        nc.scalar.activation(
                            et, sg, AF.Exp,
                            bias=(1.0 if fk > 2 * s + 1 else 0.0))
                else:
                    # DVE Schraudolph: bf16 bits = round(A*s + B)
                    eti = et.bitcast(i16)
                    if fk == 2 * s + 1:
                        nc.vector.tensor_scalar(
                            eti[:, :, 0:256], sg[:, :, 0:256],
                            SCH_A, SCH_B + SCH_A, op0=MUL, op1=ADD)
                        nc.vector.tensor_scalar(
                            eti[:, :, 256:512], sg[:, :, 256:512],
                            SCH_A, SCH_B, op0=MUL, op1=ADD)
                    else:
                        b = SCH_B + (SCH_A if fk > 2 * s + 1 else 0.0)
                        nc.vector.tensor_scalar(
                            eti, sg, SCH_A, b, op0=MUL, op1=ADD)
                if kt == 1 and pending is not None:
                    emit_norm_outproj(*pending)
                    pending = None
                if prev is not None:
                    pkt, pet = prev
                    for h in range(2):
                        nc.tensor.matmul(po[:, h, :],
                                         va[:, pkt, h, :],
                                         pet[:, h, :],
                                         start=(pkt == 0), stop=False)
                prev = (kt, et)
            pkt, pet = prev
            for h in range(2):
                nc.tensor.matmul(po[:, h, :], va[:, pkt, h, :],
                                 pet[:, h, :], start=False, stop=True)
            pending = (s, po)
        emit_norm_outproj(*pending)

        sba_ctx.close()
        ps2_ctx.close()
        ctx.close()

    nc.compile()
    return nc


def shard_inputs(x, Wqkv, bqkv, gq, gk, Wout, n_tok):
    """Build the 8 per-core input maps (head-parallel sharding)."""
    D = D_MODEL
    in_maps = []
    gq = np.asarray(gq, np.float64)
    gk = np.asarray(gk, np.float64)
    # per-partition RMS sqrt scale/bias (see build_program):
    #   q: rs = gq/8 * rsqrt(mean+eps)  -> sqs = sqrt(sumsq/gq^2 + 64eps/gq^2)
    #   k: rs = gk * rsqrt(mean+eps)    -> sqs = sqrt(sumsq/(64gk^2) + eps/gk^2)
    sq_ = np.concatenate([1.0 / gq**2, 1.0 / gq**2])
    bq_ = np.concatenate([64.0 * EPS / gq**2, 64.0 * EPS / gq**2])
    sk_ = np.concatenate([1.0 / (64.0 * gk**2), 1.0 / (64.0 * gk**2)])
    bk_ = np.concatenate([EPS / gk**2, EPS / gk**2])
    gv = np.stack([sq_, bq_, sk_, bk_], axis=1).astype(np.float32)
    for c in range(N_CORES):
        cs = slice(128 * c, 128 * (c + 1))
        wq = Wqkv[:, cs]
        wk = Wqkv[:, D + 128 * c:D + 128 * (c + 1)]
        wv = Wqkv[:, 2 * D + 128 * c:2 * D + 128 * (c + 1)]
        wqkv_s = np.ascontiguousarray(np.concatenate([wq, wk, wv], axis=1),
                                      dtype=np.float32)
        bq = bqkv[cs]
        bk = bqkv[D + 128 * c:D + 128 * (c + 1)]
        bv = bqkv[2 * D + 128 * c:2 * D + 128 * (c + 1)]
        bqkv_s = np.ascontiguousarray(np.concatenate([bq, bk, bv]),
                                      dtype=np.float32)
        wout_s = np.ascontiguousarray(Wout[cs, :], dtype=np.float32)
        in_maps.append({
            "x": np.ascontiguousarray(x[:n_tok], dtype=np.float32),
            "wqkv": wqkv_s,
            "bqkv": bqkv_s,
            "gv": np.ascontiguousarray(gv),
            "wout": wout_s,
        })
    return in_maps


_PROGRAM_CACHE = {}


def _get_program(n_tok):
    if n_tok not in _PROGRAM_CACHE:
        _PROGRAM_CACHE[n_tok] = build_program(n_tok)
    return _PROGRAM_CACHE[n_tok]


def run_sharded(inputs, trace=False, tmpdir=None):
    """Run the SPMD kernel; returns (full_output [1,N,D], BassKernelResults)."""
    from concourse.bass_utils import run_bass_kernel_spmd

    x = np.asarray(inputs["x"], dtype=np.float32)
    Wqkv = np.asarray(inputs["Wqkv"], dtype=np.float32)
    bqkv = np.asarray(inputs["bqkv"], dtype=np.float32)
    Wout = np.asarray(inputs["Wout"], dtype=np.float32)
    bout = np.asarray(inputs["bout"], dtype=np.float32)
    gq = np.asarray(inputs["gq"], dtype=np.float32)
    gk = np.asarray(inputs["gk"], dtype=np.float32)
    tpf = int(np.asarray(inputs["tokens_per_frame"]))
    assert tpf == TPF, f"kernel hardcodes tokens_per_frame={TPF}, got {tpf}"

    B, N, D = x.shape
    assert B == 1 and D == D_MODEL
    x2 = x[0]

    nc = _get_program(N)
    in_maps = shard_inputs(x2, Wqkv, bqkv, gq, gk, Wout, N)
    res = run_bass_kernel_spmd(nc, in_maps, list(range(N_CORES)),
                               trace=trace, tmpdir=tmpdir)
    acc = res.results[0]["out"].astype(np.float32)
    for c in range(1, N_CORES):
        acc = acc + res.results[c]["out"]
    if np.any(bout):
        acc = acc + bout[None, :]
    return acc[None], res


def kernel(**inputs):
    out, _ = run_sharded(inputs)
    return out


# revision 8
# speedup vs baseline: 1.2673x; 1.0182x over previous
"""Bass/Trainium2 kernel for nn_Attn_70076686401576 (block-causal-biased MHA).

Math (per reference):
  qkv = x @ Wqkv + bqkv  -> split into q,k,v heads (H=16, hd=64)
  q,k RMS-normalized over head dim (QKNorm, eps=1e-6, scales gq/gk)
  scores = q k^T / sqrt(hd) + M, where M[i,j] = 1.0 for future-frame keys
  attn = softmax(scores); o = attn @ v; out = o @ Wout + bout
Sharding: 16 heads / 8 cores = 2 heads per core (head-parallel).  Each core
computes its 2 heads' q/k/v from the full x (Wqkv column-sharded), runs full
attention for those heads, and produces a partial output via the row-sharded
Wout.  Host sums the 8 partials (+ bout).

Key structure (v2):
  - phase 1: x tiles DMA'd f32, cast to bf16 on GpSimd, transposed on PE
    (grouped 4-at-a-time into PSUM, copied out alternately by Scalar/Vector),
    projected with bf16 weights; QKNorm folds the 1/sqrt(hd) score scale and
    the gq/gk scales into the Sqrt activation's per-partition scale/bias.
  - phase 2 is a per-ktile software pipeline: the two heads' score matmuls
    run concurrently on PE row-halves into adjacent PSUM banks; the combined
    [128,2,512] tile is exponentiated by ONE engine op, alternating between
    ScalarE (exact ACT Exp) and DVE (Schraudolph bit-trick exp: one affine
    tensor_scalar emitting int16 bf16-bit-patterns) so the two engines share
    the exp load; attn@V accumulates into per-stripe PSUM; the "+1.0
    future-frame" mask is folded into the exp (ACT bias=+1 / Schraudolph
    +A_SCH) so no e-scaled V copy is needed.
  - softmax denominator via a ones-column appended to V; normalization via
    reciprocal + PE row-broadcast; output projection from bf16 oTn.
"""

import math
import numpy as np

N_TOK_FULL = 4096
D_MODEL = 1024
HD = 64
TPF = 256
EPS = 1e-6
N_CORES = 8

LN2 = math.log(2.0)
SCH_A = 128.0 / LN2            # bf16 Schraudolph multiplier
SCH_C = -6.0                   # balance constant (minimizes max rel err)
SCH_B = 127.0 * 128.0 + SCH_C


def build_program(n_tok=N_TOK_FULL, debug=False):
    import concourse.bass as bass
    import concourse.tile as tile
    from concourse import bacc, mybir
    from concourse.masks import make_identity
    from contextlib import ExitStack

    f32 = mybir.dt.float32
    f32r = mybir.dt.float32r
    bf16 = mybir.dt.bfloat16
    i16 = mybir.dt.int16
    AF = mybir.ActivationFunctionType
    MUL = mybir.AluOpType.mult
    ADD = mybir.AluOpType.add

    D = D_MODEL
    n_ranges = n_tok // 512
    n_ktiles = n_tok // 128
    n_stripes = n_tok // 512

    nc = bacc.Bacc("TRN2", target_bir_lowering=False, debug=False,
                   num_devices=N_CORES)
    x_d = nc.dram_tensor("x", [n_tok, D], f32, kind="ExternalInput").ap()
    wqkv_d = nc.dram_tensor("wqkv", [D, 384], f32, kind="ExternalInput").ap()
    bqkv_d = nc.dram_tensor("bqkv", [384], f32, kind="ExternalInput").ap()
    # gv: per-partition [scale_q, bias_q, scale_k, bias_k] for the RMS sqrt
    gv_d = nc.dram_tensor("gv", [128, 4], f32, kind="ExternalInput").ap()
    wout_d = nc.dram_tensor("wout", [128, D], f32, kind="ExternalInput").ap()
    out_d = nc.dram_tensor("out", [n_tok, D], f32, kind="ExternalOutput").ap()

    x_t = x_d.rearrange("(t p) d -> t p d", p=128)
    out_t = out_d.rearrange("(t p) d -> t p d", p=128)

    with tile.TileContext(nc) as tc:
        ctx = ExitStack()
        sb = ctx.enter_context(tc.tile_pool(name="sb", bufs=1))
        ps1_ctx = ExitStack()
        ps1 = ps1_ctx.enter_context(
            tc.tile_pool(name="ps1", bufs=1, space="PSUM"))
        sbp_ctx = ExitStack()
        sbp = sbp_ctx.enter_context(tc.tile_pool(name="sbp", bufs=1))

        # ---- constants ----
        identf = sb.tile([128, 128], f32, tag="identf")
        make_identity(nc, identf)
        identb = sb.tile([128, 128], bf16, tag="identb")
        nc.vector.tensor_copy(identb, identf)
        # block-diag ones: blkdiag.T @ sq -> per-head column sums broadcast
        # to that head's 64 partitions
        blkdf = sb.tile([128, 128], f32, tag="blkdf")
        nc.gpsimd.memset(blkdf, 0.0)
        nc.gpsimd.memset(blkdf[0:64, 0:64], 1.0)
        nc.gpsimd.memset(blkdf[64:128, 64:128], 1.0)
        blkdiag = sb.tile([128, 128], f32r, tag="blkdiag")
        nc.vector.tensor_copy(blkdiag, blkdf)
        ones64 = sb.tile([128, 64], f32, tag="ones64")
        nc.gpsimd.memset(ones64, 1.0)

        wqkvf = sb.tile([128, 8, 384], f32, tag="wqkvf")
        nc.sync.dma_start(wqkvf, wqkv_d.rearrange("(c p) n -> p c n", p=128))
        wqkv_sb = sb.tile([128, 8, 384], bf16, tag="wqkv")
        nc.vector.tensor_copy(wqkv_sb, wqkvf)
        bq_sb = sb.tile([128, 3], f32, tag="bq")
        nc.sync.dma_start(bq_sb, bqkv_d.rearrange("(c p) -> p c", p=128))
        gv_sb = sb.tile([128, 4], f32, tag="gv")
        nc.sync.dma_start(gv_sb, gv_d)
        wof = sb.tile([128, D], f32, tag="wof")
        nc.sync.dma_start(wof, wout_d)
        wo0 = sb.tile([64, D], bf16, tag="wo0")
        nc.vector.tensor_copy(wo0, wof[0:64, :])
        wo1 = sb.tile([64, D], bf16, tag="wo1")
        nc.vector.tensor_copy(wo1, wof[64:128, :])

        # ---- persistent blocks ----
        qTb = sb.tile([128, n_tok], bf16, tag="qTb")   # normalized q^T
        kTb = sb.tile([128, n_tok], bf16, tag="kTb")
        oTn0 = sb.tile([64, n_tok], bf16, tag="oTn0")
        oTn1 = sb.tile([64, n_tok], bf16, tag="oTn1")
        # V natural layout per (ktile, head): [keys=128, kt, h, hd+ones]
        va = sb.tile([128, n_ktiles, 2, 65], bf16, tag="va")
        nc.gpsimd.memset(va[:, :, :, 64:65], 1.0)

        # ================= phase 1: projection + QKNorm =================
        # stage A: transpose ALL of x into xT (bf16), DMA-overlapped
        xT = sb.tile([128, 8, n_tok], bf16, tag="xT")
        for gt in range(n_tok // 128):
            xinf = sbp.tile([128, D], f32, tag="xinf", bufs=4)
            nc.sync.dma_start(xinf[0:64, :], x_t[gt][0:64, :])
            nc.sync.dma_start(xinf[64:128, :], x_t[gt][64:128, :])
            xin = sbp.tile([128, D], bf16, tag="xin", bufs=4)
            if gt % 2 == 0:
                nc.scalar.copy(xin, xinf)
            else:
                nc.vector.tensor_copy(xin, xinf)
            xp = ps1.tile([128, 8, 128], bf16, tag="xp", bufs=3,
                          name=f"xp_{gt}")
            for dc in range(8):
                nc.tensor.transpose(
                    xp[:, dc, :], xin[:, dc * 128:(dc + 1) * 128],
                    identb)
            dst = xT[:, :, gt * 128:(gt + 1) * 128]
            if gt % 2 == 0:
                nc.vector.tensor_copy(dst, xp)
            else:
                nc.scalar.copy(dst, xp)

        # stage B: projection + QKNorm + V prep, PE-dense
        for r in range(n_ranges):
            pj = ps1.tile([128, 3, 512], f32, tag="pj", bufs=1, name=f"pj_{r}")
            for dc in range(8):
                for oc in range(3):
                    nc.tensor.matmul(
                        pj[:, oc, :],
                        wqkv_sb[:, dc, oc * 128:(oc + 1) * 128],
                        xT[:, dc, r * 512:(r + 1) * 512],
                        start=(dc == 0), stop=(dc == 7))
            sl = slice(r * 512, (r + 1) * 512)
            qTr = sbp.tile([128, 512], f32r, tag="qTr", bufs=2)
            kTr = sbp.tile([128, 512], f32r, tag="kTr", bufs=2)
            vTr = sbp.tile([128, 512], bf16, tag="vTr", bufs=2)
            nc.vector.tensor_scalar_add(qTr, pj[:, 0, :], bq_sb[:, 0:1])
            nc.vector.tensor_scalar_add(kTr, pj[:, 1, :], bq_sb[:, 1:2])
            nc.vector.tensor_scalar_add(vTr, pj[:, 2, :], bq_sb[:, 2:3])

            # QKNorm: rs = g * rsqrt(mean(t^2) + eps); q additionally folds
            # the 1/sqrt(hd) score scale.  scale/bias of the Sqrt activation
            # are per-partition host-precomputed: sqs = sqrt(sumsq*s + b),
            # rs = 1/sqs.
            for blk, blkb, gcol in ((qTr, qTb, 0), (kTr, kTb, 1)):
                sq = sbp.tile([128, 512], f32r, tag="sq", bufs=2)
                nc.scalar.activation(sq, blk, AF.Square)
                ps_r = ps1.tile([128, 512], f32, tag="psr", bufs=2,
                                name=f"psr_{r}_{gcol}")
                nc.tensor.matmul(ps_r, blkdiag, sq, start=True, stop=True)
                sqs = sbp.tile([128, 512], f32, tag="sqs", bufs=2)
                nc.scalar.activation(sqs, ps_r, AF.Sqrt,
                                     bias=gv_sb[:, 2 * gcol + 1:2 * gcol + 2],
                                     scale=gv_sb[:, 2 * gcol:2 * gcol + 1])
                rs = sbp.tile([128, 512], f32, tag="rs", bufs=2)
                nc.vector.reciprocal_approx_fast(rs, sqs)
                nc.vector.tensor_mul(blkb[:, sl], blk, rs)

            # V -> va for this range's 4 ktiles
            vp = ps1.tile([128, 8, 128], bf16, tag="xp", bufs=3,
                          name=f"vp_{r}")
            for q in range(4):
                nc.tensor.transpose(
                    vp[:, q, :], vTr[:, q * 128:(q + 1) * 128], identb)
            src = vp[:, 0:4, :].rearrange("p k (h d) -> p k h d", h=2)
            if r % 2 == 0:
                nc.scalar.copy(va[:, 4 * r:4 * r + 4, :, 0:64], src)
            else:
                nc.vector.tensor_copy(va[:, 4 * r:4 * r + 4, :, 0:64], src)

        # ================= phase 2: attention =================
        sbp_ctx.close()
        ps1_ctx.close()
        ps2_ctx = ExitStack()
        ps2 = ps2_ctx.enter_context(
            tc.tile_pool(name="ps2", bufs=1, space="PSUM"))
        sba_ctx = ExitStack()
        sba = sba_ctx.enter_context(tc.tile_pool(name="sba", bufs=1))

        def make_norm_steps(s, po):
            """Normalize stripe s + output proj, as a list of emission steps
            to be spread across the next stripe's kt loop (keeps engine
            FIFOs shallow so PE never gaps at stripe boundaries)."""
            qsl = slice(s * 512, (s + 1) * 512)
            st = {}

            def step_recip():
                # custom-DVE ops misbehave at base_partition != 0: compute
                # recip over all 65 rows from base 0; only row 64 (the
                # denominator) is consumed by the broadcast matmul
                rd = sba.tile([65, 2, 512], f32, tag="rd", bufs=2,
                              name=f"rd_{s}")
                nc.vector.reciprocal_approx_fast(rd, po)
                st["rd"] = rd

            def step_bcast():
                ps_b = ps2.tile([128, 2, 512], f32, tag="sg", bufs=2,
                                name=f"psb_{s}")
                for h in range(2):
                    nc.tensor.matmul(ps_b[0:64, h, :],
                                     ones64[64:65, :],
                                     st["rd"][64:65, h, :],
                                     start=True, stop=True,
                                     tile_position=(64, 0))
                st["psb"] = ps_b

            def step_rb():
                rb = sba.tile([64, 2, 512], f32, tag="rb", bufs=2,
                              name=f"rb_{s}")
                nc.scalar.copy(rb, st["psb"][0:64, :, :])
                st["rb"] = rb

            def step_mul():
                for h, oTn in ((0, oTn0), (1, oTn1)):
                    nc.vector.tensor_mul(oTn[:, qsl], po[0:64, h, :],
                                         st["rb"][:, h, :])

            def make_proj(tt):
                def step_proj():
                    t0 = s * 512 + tt * 128
                    ps_o = ps2.tile([128, 2, 512], f32, tag="sg", bufs=2,
                                    name=f"pso_{s}_{tt}")
                    for half in range(2):
                        nsl = slice(half * 512, (half + 1) * 512)
                        nc.tensor.matmul(ps_o[:, half, :],
                                         oTn0[:, t0:t0 + 128],
                                         wo0[:, nsl],
                                         start=True, stop=False)
                        nc.tensor.matmul(ps_o[:, half, :],
                                         oTn1[:, t0:t0 + 128],
                                         wo1[:, nsl],
                                         start=False, stop=True)
                    ob = sba.tile([128, 2, 512], f32, tag="ob", bufs=3,
                                  name=f"ob_{s}_{tt}")
                    if tt == 3:
                        nc.vector.tensor_copy(ob, ps_o)
                    else:
                        nc.scalar.copy(ob, ps_o)
                    nc.sync.dma_start(out_t[t0 // 128], ob)
                return step_proj

            return [step_recip, step_bcast, step_rb, step_mul,
                    make_proj(0), make_proj(1), make_proj(2), make_proj(3)]

        # norm steps of stripe s-1 are emitted at these kt indices of stripe s
        STEP_KTS = {2: 0, 4: 1, 6: 2, 8: 3, 10: 4, 12: 5, 14: 6, 16: 7}

        pending = None
        for s in range(n_stripes):
            qsl = slice(s * 512, (s + 1) * 512)
            po = ps2.tile([65, 2, 512], f32, tag="po", bufs=2,
                          name=f"po_{s}")
            prev = None
            for kt in range(n_ktiles):
                sg = ps2.tile([128, 2, 512], f32, tag="sg", bufs=2,
                              name=f"sg_{s}_{kt}")
                for h in range(2):
                    hp = slice(h * 64, (h + 1) * 64)
                    nc.tensor.matmul(
                        sg[:, h, :],
                        kTb[hp, kt * 128:(kt + 1) * 128],
                        qTb[hp, qsl],
                        start=True, stop=True,
                        tile_position=(h * 64, 0))
                # exp (mask folded in): key frame fk vs query frames
                # (2s, 2s+1): future -> +1.0 bias
                et = sba.tile([128, 2, 512], bf16, tag="et", bufs=5,
                              name=f"et_{s}_{kt}")
                fk = kt // 2
                if kt % 2 == 0:
                    # ScalarE exact exp
                    if fk == 2 * s + 1:
                        nc.scalar.activation(et[:, :, 0:256],
                                             sg[:, :, 0:256], AF.Exp,
                                             bias=1.0)
                        nc.scalar.activation(et[:, :, 256:512],
                                             sg[:, :, 256:512], AF.Exp)
                    else:
                        nc.scalar.activation(
                            et, sg, AF.Exp,
                            bias=(1.0 if fk > 2 * s + 1 else 0.0))
                else:
                    # DVE Schraudolph: bf16 bits = round(A*s + B)
                    eti = et.bitcast(i16)
                    if fk == 2 * s + 1:
                        nc.vector.tensor_scalar(
                            eti[:, :, 0:256], sg[:, :, 0:256],
                            SCH_A, SCH_B + SCH_A, op0=MUL, op1=ADD)
                        nc.vector.tensor_scalar(
                            eti[:, :, 256:512], sg[:, :, 256:512],
                            SCH_A, SCH_B, op0=MUL, op1=ADD)
                    else:
                        b = SCH_B + (SCH_A if fk > 2 * s + 1 else 0.0)
                        nc.vector.tensor_scalar(
                            eti, sg, SCH_A, b, op0=MUL, op1=ADD)
                if pending is not None and kt in STEP_KTS:
                    pending[STEP_KTS[kt]]()
                    if STEP_KTS[kt] == len(pending) - 1:
                        pending = None
                if prev is not None:
                    pkt, pet = prev
                    for h in range(2):
                        nc.tensor.matmul(po[:, h, :],
                                         va[:, pkt, h, :],
                                         pet[:, h, :],
                                         start=(pkt == 0), stop=False)
                prev = (kt, et)
            pkt, pet = prev
            for h in range(2):
                nc.tensor.matmul(po[:, h, :], va[:, pkt, h, :],
                                 pet[:, h, :], start=False, stop=True)
            pending = make_norm_steps(s, po)
        for step in pending:
            step()

        sba_ctx.close()
        ps2_ctx.close()
        ctx.close()

    nc.compile()
    return nc


def shard_inputs(x, Wqkv, bqkv, gq, gk, Wout, n_tok):
    """Build the 8 per-core input maps (head-parallel sharding)."""
    D = D_MODEL
    in_maps = []
    gq = np.asarray(gq, np.float64)
    gk = np.asarray(gk, np.float64)
    # per-partition RMS sqrt scale/bias (see build_program):
    #   q: rs = gq/8 * rsqrt(mean+eps)  -> sqs = sqrt(sumsq/gq^2 + 64eps/gq^2)
    #   k: rs = gk * rsqrt(mean+eps)    -> sqs = sqrt(sumsq/(64gk^2) + eps/gk^2)
    sq_ = np.concatenate([1.0 / gq**2, 1.0 / gq**2])
    bq_ = np.concatenate([64.0 * EPS / gq**2, 64.0 * EPS / gq**2])
    sk_ = np.concatenate([1.0 / (64.0 * gk**2), 1.0 / (64.0 * gk**2)])
    bk_ = np.concatenate([EPS / gk**2, EPS / gk**2])
    gv = np.stack([sq_, bq_, sk_, bk_], axis=1).astype(np.float32)
    for c in range(N_CORES):
        cs = slice(128 * c, 128 * (c + 1))
        wq = Wqkv[:, cs]
        wk = Wqkv[:, D + 128 * c:D + 128 * (c + 1)]
        wv = Wqkv[:, 2 * D + 128 * c:2 * D + 128 * (c + 1)]
        wqkv_s = np.ascontiguousarray(np.concatenate([wq, wk, wv], axis=1),
                                      dtype=np.float32)
        bq = bqkv[cs]
        bk = bqkv[D + 128 * c:D + 128 * (c + 1)]
        bv = bqkv[2 * D + 128 * c:2 * D + 128 * (c + 1)]
        bqkv_s = np.ascontiguousarray(np.concatenate([bq, bk, bv]),
                                      dtype=np.float32)
        wout_s = np.ascontiguousarray(Wout[cs, :], dtype=np.float32)
        in_maps.append({
            "x": np.ascontiguousarray(x[:n_tok], dtype=np.float32),
            "wqkv": wqkv_s,
            "bqkv": bqkv_s,
            "gv": np.ascontiguousarray(gv),
            "wout": wout_s,
        })
    return in_maps


_PROGRAM_CACHE = {}


def _get_program(n_tok):
    if n_tok not in _PROGRAM_CACHE:
        _PROGRAM_CACHE[n_tok] = build_program(n_tok)
    return _PROGRAM_CACHE[n_tok]


def run_sharded(inputs, trace=False, tmpdir=None):
    """Run the SPMD kernel; returns (full_output [1,N,D], BassKernelResults)."""
    from concourse.bass_utils import run_bass_kernel_spmd

    x = np.asarray(inputs["x"], dtype=np.float32)
    Wqkv = np.asarray(inputs["Wqkv"], dtype=np.float32)
    bqkv = np.asarray(inputs["bqkv"], dtype=np.float32)
    Wout = np.asarray(inputs["Wout"], dtype=np.float32)
    bout = np.asarray(inputs["bout"], dtype=np.float32)
    gq = np.asarray(inputs["gq"], dtype=np.float32)
    gk = np.asarray(inputs["gk"], dtype=np.float32)
    tpf = int(np.asarray(inputs["tokens_per_frame"]))
    assert tpf == TPF, f"kernel hardcodes tokens_per_frame={TPF}, got {tpf}"

    B, N, D = x.shape
    assert B == 1 and D == D_MODEL
    x2 = x[0]

    nc = _get_program(N)
    in_maps = shard_inputs(x2, Wqkv, bqkv, gq, gk, Wout, N)
    res = run_bass_kernel_spmd(nc, in_maps, list(range(N_CORES)),
                               trace=trace, tmpdir=tmpdir)
    acc = res.results[0]["out"].astype(np.float32)
    for c in range(1, N_CORES):
        acc = acc + res.results[c]["out"]
    if np.any(bout):
        acc = acc + bout[None, :]
    return acc[None], res


def kernel(**inputs):
    out, _ = run_sharded(inputs)
    return out


# revision 10
# speedup vs baseline: 1.4378x; 1.1345x over previous
"""Bass/Trainium2 kernel for nn_Attn_70076686401576 (block-causal-biased MHA).

Math (per reference):
  qkv = x @ Wqkv + bqkv  -> split into q,k,v heads (H=16, hd=64)
  q,k RMS-normalized over head dim (QKNorm, eps=1e-6, scales gq/gk)
  scores = q k^T / sqrt(hd) + M, where M[i,j] = 1.0 for future-frame keys
  attn = softmax(scores); o = attn @ v; out = o @ Wout + bout
Sharding: 16 heads / 8 cores = 2 heads per core (head-parallel).  Each core
computes its 2 heads' q/k/v from the full x (Wqkv column-sharded), runs full
attention for those heads, and produces a partial output via the row-sharded
Wout.  Host sums the 8 partials (+ bout).

Key structure (v2):
  - phase 1: x tiles DMA'd f32, cast to bf16 on GpSimd, transposed on PE
    (grouped 4-at-a-time into PSUM, copied out alternately by Scalar/Vector),
    projected with bf16 weights; QKNorm folds the 1/sqrt(hd) score scale and
    the gq/gk scales into the Sqrt activation's per-partition scale/bias.
  - phase 2 is a per-ktile software pipeline: the two heads' score matmuls
    run concurrently on PE row-halves into adjacent PSUM banks; the combined
    [128,2,512] tile is exponentiated by ONE engine op, alternating between
    ScalarE (exact ACT Exp) and DVE (Schraudolph bit-trick exp: one affine
    tensor_scalar emitting int16 bf16-bit-patterns) so the two engines share
    the exp load; attn@V accumulates into per-stripe PSUM; the "+1.0
    future-frame" mask is folded into the exp (ACT bias=+1 / Schraudolph
    +A_SCH) so no e-scaled V copy is needed.
  - softmax denominator via a ones-column appended to V; normalization via
    reciprocal + PE row-broadcast; output projection from bf16 oTn.
"""

import math
import numpy as np

N_TOK_FULL = 4096
D_MODEL = 1024
HD = 64
TPF = 256
EPS = 1e-6
N_CORES = 8

LN2 = math.log(2.0)
SCH_A = 128.0 / LN2            # bf16 Schraudolph multiplier
SCH_C = -6.0                   # balance constant (minimizes max rel err)
SCH_B = 127.0 * 128.0 + SCH_C


def build_program(n_tok=N_TOK_FULL, debug=False):
    import concourse.bass as bass
    import concourse.tile as tile
    from concourse import bacc, mybir
    from concourse.masks import make_identity
    from contextlib import ExitStack

    f32 = mybir.dt.float32
    f32r = mybir.dt.float32r
    bf16 = mybir.dt.bfloat16
    i16 = mybir.dt.int16
    AF = mybir.ActivationFunctionType
    MUL = mybir.AluOpType.mult
    ADD = mybir.AluOpType.add

    D = D_MODEL
    n_ranges = n_tok // 512
    n_ktiles = n_tok // 128
    n_stripes = n_tok // 512

    nc = bacc.Bacc("TRN2", target_bir_lowering=False, debug=False,
                   num_devices=N_CORES)
    x_d = nc.dram_tensor("x", [n_tok, D], f32, kind="ExternalInput").ap()
    wqkv_d = nc.dram_tensor("wqkv", [D, 384], f32, kind="ExternalInput").ap()
    bqkv_d = nc.dram_tensor("bqkv", [384], f32, kind="ExternalInput").ap()
    # gv: per-partition [scale_q, bias_q, scale_k, bias_k] for the RMS sqrt
    gv_d = nc.dram_tensor("gv", [128, 4], f32, kind="ExternalInput").ap()
    wout_d = nc.dram_tensor("wout", [128, D], f32, kind="ExternalInput").ap()
    out_d = nc.dram_tensor("out", [n_tok, D], f32, kind="ExternalOutput").ap()

    x_t = x_d.rearrange("(t p) d -> t p d", p=128)
    out_t = out_d.rearrange("(t p) d -> t p d", p=128)

    with tile.TileContext(nc) as tc:
        ctx = ExitStack()
        sb = ctx.enter_context(tc.tile_pool(name="sb", bufs=1))
        ps1_ctx = ExitStack()
        ps1 = ps1_ctx.enter_context(
            tc.tile_pool(name="ps1", bufs=1, space="PSUM"))
        sbp_ctx = ExitStack()
        sbp = sbp_ctx.enter_context(tc.tile_pool(name="sbp", bufs=1))

        # ---- constants ----
        identf = sb.tile([128, 128], f32, tag="identf")
        make_identity(nc, identf)
        identb = sb.tile([128, 128], bf16, tag="identb")
        nc.vector.tensor_copy(identb, identf)
        # block-diag ones: blkdiag.T @ sq -> per-head column sums broadcast
        # to that head's 64 partitions
        blkdf = sb.tile([128, 128], f32, tag="blkdf")
        nc.gpsimd.memset(blkdf, 0.0)
        nc.gpsimd.memset(blkdf[0:64, 0:64], 1.0)
        nc.gpsimd.memset(blkdf[64:128, 64:128], 1.0)
        blkdiag = sb.tile([128, 128], f32r, tag="blkdiag")
        nc.vector.tensor_copy(blkdiag, blkdf)
        ones64 = sb.tile([128, 64], f32, tag="ones64")
        nc.gpsimd.memset(ones64, 1.0)

        wqkvf = sb.tile([128, 8, 384], f32, tag="wqkvf")
        nc.sync.dma_start(wqkvf, wqkv_d.rearrange("(c p) n -> p c n", p=128))
        wqkv_sb = sb.tile([128, 8, 384], bf16, tag="wqkv")
        nc.vector.tensor_copy(wqkv_sb, wqkvf)
        bq_sb = sb.tile([128, 3], f32, tag="bq")
        nc.sync.dma_start(bq_sb, bqkv_d.rearrange("(c p) -> p c", p=128))
        gv_sb = sb.tile([128, 4], f32, tag="gv")
        nc.sync.dma_start(gv_sb, gv_d)
        wof = sb.tile([128, D], f32, tag="wof")
        nc.sync.dma_start(wof, wout_d)
        wo0 = sb.tile([64, D], bf16, tag="wo0")
        nc.vector.tensor_copy(wo0, wof[0:64, :])
        wo1 = sb.tile([64, D], bf16, tag="wo1")
        nc.vector.tensor_copy(wo1, wof[64:128, :])

        # ---- persistent blocks ----
        qTb = sb.tile([128, n_tok], bf16, tag="qTb")   # normalized q^T
        kTb = sb.tile([128, n_tok], bf16, tag="kTb")
        oTn0 = sb.tile([64, n_tok], bf16, tag="oTn0")
        oTn1 = sb.tile([64, n_tok], bf16, tag="oTn1")
        # V natural layout per (ktile, head): [keys=128, kt, h, hd+ones]
        va = sb.tile([128, n_ktiles, 2, 65], bf16, tag="va")
        nc.gpsimd.memset(va[:, :, :, 64:65], 1.0)

        # ================= phase 1: projection + QKNorm =================
        # stage A: transpose ALL of x into xT (bf16), DMA-overlapped
        xT = sb.tile([128, 8, n_tok], bf16, tag="xT")
        for gt in range(n_tok // 128):
            xinf = sbp.tile([128, D], f32, tag="xinf", bufs=4)
            nc.sync.dma_start(xinf[0:64, :], x_t[gt][0:64, :])
            nc.sync.dma_start(xinf[64:128, :], x_t[gt][64:128, :])
            xin = sbp.tile([128, D], bf16, tag="xin", bufs=4)
            if gt % 2 == 0:
                nc.scalar.copy(xin, xinf)
            else:
                nc.vector.tensor_copy(xin, xinf)
            xp = ps1.tile([128, 8, 128], bf16, tag="xp", bufs=3,
                          name=f"xp_{gt}")
            for dc in range(8):
                nc.tensor.transpose(
                    xp[:, dc, :], xin[:, dc * 128:(dc + 1) * 128],
                    identb)
            dst = xT[:, :, gt * 128:(gt + 1) * 128]
            if gt % 2 == 0:
                nc.vector.tensor_copy(dst, xp)
            else:
                nc.scalar.copy(dst, xp)

        # stage B: projection + QKNorm + V prep, PE-dense
        for r in range(n_ranges):
            pj = ps1.tile([128, 3, 512], f32, tag="pj", bufs=1, name=f"pj_{r}")
            for dc in range(8):
                for oc in range(3):
                    nc.tensor.matmul(
                        pj[:, oc, :],
                        wqkv_sb[:, dc, oc * 128:(oc + 1) * 128],
                        xT[:, dc, r * 512:(r + 1) * 512],
                        start=(dc == 0), stop=(dc == 7))
            sl = slice(r * 512, (r + 1) * 512)
            qTr = sbp.tile([128, 512], f32r, tag="qTr", bufs=2)
            kTr = sbp.tile([128, 512], f32r, tag="kTr", bufs=2)
            vTr = sbp.tile([128, 512], bf16, tag="vTr", bufs=2)
            nc.vector.tensor_scalar_add(qTr, pj[:, 0, :], bq_sb[:, 0:1])
            nc.vector.tensor_scalar_add(kTr, pj[:, 1, :], bq_sb[:, 1:2])
            nc.vector.tensor_scalar_add(vTr, pj[:, 2, :], bq_sb[:, 2:3])

            # QKNorm: rs = g * rsqrt(mean(t^2) + eps); q additionally folds
            # the 1/sqrt(hd) score scale.  scale/bias of the Sqrt activation
            # are per-partition host-precomputed: sqs = sqrt(sumsq*s + b),
            # rs = 1/sqs.
            for blk, blkb, gcol in ((qTr, qTb, 0), (kTr, kTb, 1)):
                sq = sbp.tile([128, 512], f32r, tag="sq", bufs=2)
                nc.scalar.activation(sq, blk, AF.Square)
                ps_r = ps1.tile([128, 512], f32, tag="psr", bufs=2,
                                name=f"psr_{r}_{gcol}")
                nc.tensor.matmul(ps_r, blkdiag, sq, start=True, stop=True)
                sqs = sbp.tile([128, 512], f32, tag="sqs", bufs=2)
                nc.scalar.activation(sqs, ps_r, AF.Sqrt,
                                     bias=gv_sb[:, 2 * gcol + 1:2 * gcol + 2],
                                     scale=gv_sb[:, 2 * gcol:2 * gcol + 1])
                rs = sbp.tile([128, 512], f32, tag="rs", bufs=2)
                nc.vector.reciprocal_approx_fast(rs, sqs)
                nc.vector.tensor_mul(blkb[:, sl], blk, rs)

            # V -> va for this range's 4 ktiles
            vp = ps1.tile([128, 8, 128], bf16, tag="xp", bufs=3,
                          name=f"vp_{r}")
            for q in range(4):
                nc.tensor.transpose(
                    vp[:, q, :], vTr[:, q * 128:(q + 1) * 128], identb)
            src = vp[:, 0:4, :].rearrange("p k (h d) -> p k h d", h=2)
            if r % 2 == 0:
                nc.scalar.copy(va[:, 4 * r:4 * r + 4, :, 0:64], src)
            else:
                nc.vector.tensor_copy(va[:, 4 * r:4 * r + 4, :, 0:64], src)

        # ================= phase 2: attention =================
        sbp_ctx.close()
        ps1_ctx.close()
        ps2_ctx = ExitStack()
        ps2 = ps2_ctx.enter_context(
            tc.tile_pool(name="ps2", bufs=1, space="PSUM"))
        sba_ctx = ExitStack()
        sba = sba_ctx.enter_context(tc.tile_pool(name="sba", bufs=1))

        def make_norm_steps(s, po):
            """Normalize stripe s + output proj, as a list of emission steps
            to be spread across the next stripe's kt loop (keeps engine
            FIFOs shallow so PE never gaps at stripe boundaries)."""
            qsl = slice(s * 512, (s + 1) * 512)
            st = {}

            def step_recip():
                # custom-DVE ops misbehave at base_partition != 0: compute
                # recip over all 65 rows from base 0; only row 64 (the
                # denominator) is consumed by the broadcast matmul
                rd = sba.tile([65, 2, 512], f32, tag="rd", bufs=2,
                              name=f"rd_{s}")
                nc.vector.reciprocal_approx_fast(rd, po)
                st["rd"] = rd

            def step_bcast():
                ps_b = ps2.tile([128, 2, 512], f32, tag="sg", bufs=2,
                                name=f"psb_{s}")
                for h in range(2):
                    nc.tensor.matmul(ps_b[0:64, h, :],
                                     ones64[64:65, :],
                                     st["rd"][64:65, h, :],
                                     start=True, stop=True,
                                     tile_position=(64, 0))
                st["psb"] = ps_b

            def step_rb():
                rb = sba.tile([64, 2, 512], f32, tag="rb", bufs=2,
                              name=f"rb_{s}")
                nc.scalar.copy(rb, st["psb"][0:64, :, :])
                st["rb"] = rb

            def step_mul():
                for h, oTn in ((0, oTn0), (1, oTn1)):
                    nc.vector.tensor_mul(oTn[:, qsl], po[0:64, h, :],
                                         st["rb"][:, h, :])

            def make_proj(tt):
                def step_proj():
                    t0 = s * 512 + tt * 128
                    ps_o = ps2.tile([128, 2, 512], f32, tag="sg", bufs=2,
                                    name=f"pso_{s}_{tt}")
                    for half in range(2):
                        nsl = slice(half * 512, (half + 1) * 512)
                        nc.tensor.matmul(ps_o[:, half, :],
                                         oTn0[:, t0:t0 + 128],
                                         wo0[:, nsl],
                                         start=True, stop=False)
                        nc.tensor.matmul(ps_o[:, half, :],
                                         oTn1[:, t0:t0 + 128],
                                         wo1[:, nsl],
                                         start=False, stop=True)
                    ob = sba.tile([128, 2, 512], f32, tag="ob", bufs=3,
                                  name=f"ob_{s}_{tt}")
                    if tt == 3:
                        nc.vector.tensor_copy(ob, ps_o)
                    else:
                        nc.scalar.copy(ob, ps_o)
                    nc.sync.dma_start(out_t[t0 // 128], ob)
                return step_proj

            return [step_recip, step_bcast, step_rb, step_mul,
                    make_proj(0), make_proj(1), make_proj(2), make_proj(3)]

        # norm steps of stripe s-1 are emitted at these kt indices of stripe s
        STEP_KTS = {2: 0, 4: 1, 6: 2, 8: 3, 10: 4, 12: 5, 14: 6, 16: 7}

        LAG = 2  # attn@V trails scores/exp by LAG ktiles so PE never waits
        pending = None
        for s in range(n_stripes):
            qsl = slice(s * 512, (s + 1) * 512)
            po = ps2.tile([65, 2, 512], f32, tag="po", bufs=2,
                          name=f"po_{s}")
            prev = []
            for kt in range(n_ktiles):
                sg = ps2.tile([128, 2, 512], f32, tag="sg", bufs=2,
                              name=f"sg_{s}_{kt}")
                for h in range(2):
                    hp = slice(h * 64, (h + 1) * 64)
                    nc.tensor.matmul(
                        sg[:, h, :],
                        kTb[hp, kt * 128:(kt + 1) * 128],
                        qTb[hp, qsl],
                        start=True, stop=True,
                        tile_position=(h * 64, 0))
                # exp (mask folded in): key frame fk vs query frames
                # (2s, 2s+1): future -> +1.0 bias
                et = sba.tile([128, 2, 512], bf16, tag="et", bufs=5,
                              name=f"et_{s}_{kt}")
                fk = kt // 2
                if kt % 2 == 0:
                    # ScalarE exact exp
                    if fk == 2 * s + 1:
                        nc.scalar.activation(et[:, :, 0:256],
                                             sg[:, :, 0:256], AF.Exp,
                                             bias=1.0)
                        nc.scalar.activation(et[:, :, 256:512],
                                             sg[:, :, 256:512], AF.Exp)
                    else:
                        nc.scalar.activation(
                            et, sg, AF.Exp,
                            bias=(1.0 if fk > 2 * s + 1 else 0.0))
                else:
                    # DVE Schraudolph: bf16 bits = round(A*s + B)
                    eti = et.bitcast(i16)
                    if fk == 2 * s + 1:
                        nc.vector.tensor_scalar(
                            eti[:, :, 0:256], sg[:, :, 0:256],
                            SCH_A, SCH_B + SCH_A, op0=MUL, op1=ADD)
                        nc.vector.tensor_scalar(
                            eti[:, :, 256:512], sg[:, :, 256:512],
                            SCH_A, SCH_B, op0=MUL, op1=ADD)
                    else:
                        b = SCH_B + (SCH_A if fk > 2 * s + 1 else 0.0)
                        nc.vector.tensor_scalar(
                            eti, sg, SCH_A, b, op0=MUL, op1=ADD)
                if pending is not None and kt in STEP_KTS:
                    pending[STEP_KTS[kt]]()
                    if STEP_KTS[kt] == len(pending) - 1:
                        pending = None
                prev.append((kt, et))
                if len(prev) > LAG:
                    pkt, pet = prev.pop(0)
                    for h in range(2):
                        nc.tensor.matmul(po[:, h, :],
                                         va[:, pkt, h, :],
                                         pet[:, h, :],
                                         start=(pkt == 0), stop=False)
            for pkt, pet in prev:
                for h in range(2):
                    nc.tensor.matmul(po[:, h, :], va[:, pkt, h, :],
                                     pet[:, h, :],
                                     start=False, stop=(pkt == n_ktiles - 1))
            prev = []
            pending = make_norm_steps(s, po)
        for step in pending:
            step()

        sba_ctx.close()
        ps2_ctx.close()
        ctx.close()

    nc.compile()
    return nc


def shard_inputs(x, Wqkv, bqkv, gq, gk, Wout, n_tok):
    """Build the 8 per-core input maps (head-parallel sharding)."""
    D = D_MODEL
    in_maps = []
    gq = np.asarray(gq, np.float64)
    gk = np.asarray(gk, np.float64)
    # per-partition RMS sqrt scale/bias (see build_program):
    #   q: rs = gq/8 * rsqrt(mean+eps)  -> sqs = sqrt(sumsq/gq^2 + 64eps/gq^2)
    #   k: rs = gk * rsqrt(mean+eps)    -> sqs = sqrt(sumsq/(64gk^2) + eps/gk^2)
    sq_ = np.concatenate([1.0 / gq**2, 1.0 / gq**2])
    bq_ = np.concatenate([64.0 * EPS / gq**2, 64.0 * EPS / gq**2])
    sk_ = np.concatenate([1.0 / (64.0 * gk**2), 1.0 / (64.0 * gk**2)])
    bk_ = np.concatenate([EPS / gk**2, EPS / gk**2])
    gv = np.stack([sq_, bq_, sk_, bk_], axis=1).astype(np.float32)
    for c in range(N_CORES):
        cs = slice(128 * c, 128 * (c + 1))
        wq = Wqkv[:, cs]
        wk = Wqkv[:, D + 128 * c:D + 128 * (c + 1)]
        wv = Wqkv[:, 2 * D + 128 * c:2 * D + 128 * (c + 1)]
        wqkv_s = np.ascontiguousarray(np.concatenate([wq, wk, wv], axis=1),
                                      dtype=np.float32)
        bq = bqkv[cs]
        bk = bqkv[D + 128 * c:D + 128 * (c + 1)]
        bv = bqkv[2 * D + 128 * c:2 * D + 128 * (c + 1)]
        bqkv_s = np.ascontiguousarray(np.concatenate([bq, bk, bv]),
                                      dtype=np.float32)
        wout_s = np.ascontiguousarray(Wout[cs, :], dtype=np.float32)
        in_maps.append({
            "x": np.ascontiguousarray(x[:n_tok], dtype=np.float32),
            "wqkv": wqkv_s,
            "bqkv": bqkv_s,
            "gv": np.ascontiguousarray(gv),
            "wout": wout_s,
        })
    return in_maps


_PROGRAM_CACHE = {}


def _get_program(n_tok):
    if n_tok not in _PROGRAM_CACHE:
        _PROGRAM_CACHE[n_tok] = build_program(n_tok)
    return _PROGRAM_CACHE[n_tok]


def run_sharded(inputs, trace=False, tmpdir=None):
    """Run the SPMD kernel; returns (full_output [1,N,D], BassKernelResults)."""
    from concourse.bass_utils import run_bass_kernel_spmd

    x = np.asarray(inputs["x"], dtype=np.float32)
    Wqkv = np.asarray(inputs["Wqkv"], dtype=np.float32)
    bqkv = np.asarray(inputs["bqkv"], dtype=np.float32)
    Wout = np.asarray(inputs["Wout"], dtype=np.float32)
    bout = np.asarray(inputs["bout"], dtype=np.float32)
    gq = np.asarray(inputs["gq"], dtype=np.float32)
    gk = np.asarray(inputs["gk"], dtype=np.float32)
    tpf = int(np.asarray(inputs["tokens_per_frame"]))
    assert tpf == TPF, f"kernel hardcodes tokens_per_frame={TPF}, got {tpf}"

    B, N, D = x.shape
    assert B == 1 and D == D_MODEL
    x2 = x[0]

    nc = _get_program(N)
    in_maps = shard_inputs(x2, Wqkv, bqkv, gq, gk, Wout, N)
    res = run_bass_kernel_spmd(nc, in_maps, list(range(N_CORES)),
                               trace=trace, tmpdir=tmpdir)
    acc = res.results[0]["out"].astype(np.float32)
    for c in range(1, N_CORES):
        acc = acc + res.results[c]["out"]
    if np.any(bout):
        acc = acc + bout[None, :]
    return acc[None], res


def kernel(**inputs):
    out, _ = run_sharded(inputs)
    return out
